# revision 11
# baseline (speedup 1.0000x reference)
"""Trainium2 Bass kernel for the PK-batch message-passing gating module.

Reference computation (per full batch of N=80 samples, 8 identities x
(5 sub=1 + 5 sub=0) samples):
  for each branch b in {sub==1, sub==0}:
    xs   = Wr_b @ x[subgroup_b]                (1x1 conv 2048 -> 256)
    aff  = per-sample gather of the 5 same-label subgroup samples,
           channel-stacked -> 1280 channels
    s_b  = relu(Wc_b @ aff)                    (1x1 conv 1280 -> 2048)
  x_fuse = sigmoid(W_f @ concat(s_i, s_v))     (1x1 conv 4096 -> 2048)
  out    = inputs * (1 + x_fuse)

All samples of one identity share the same gather, hence the same
x_fuse — so the message passing is computed once per identity and the
sigmoid gate broadcast over that identity's 10 samples.  Sharding: one
identity per NeuronCore (8 identities / 8 cores, data parallel);
weights replicated, pre-transposed/tiled on host, fp8 e3m4.

x ships ONCE as bf16 and feeds both the stage-R matmuls (bf16 moving
data runs at the same 1 col/cycle as fp8) and the output gate — no
separate fp8 copy, which both cuts HBM traffic ~11% and removes the
x-quantization error.  The single-ring FIFO DMA order is the schedule:
wr + x chunks first (stage R streams against them), the first two
expand-conv pairs slipped in before the last x chunks so stage E is
never starved, then the rest of wc and all of wf — everything on-chip
before its stage begins, so the F window carries only output stores
and the store tail after the last matmul is just the final chunk.
"""

import numpy as np
import ml_dtypes

import concourse.bass as bass
import concourse.tile as tile
from concourse import bacc, mybir
from concourse.bass_utils import run_bass_kernel_spmd

N_CORES = 8
K_HALF = 5
NSAMP = 2 * K_HALF        # samples per identity
DIM = 2048
CP = 256                  # reduced channels per branch
S = 24 * 8                # spatial positions per sample
NT = DIM // 128           # 16 channel chunks of the 2048-dim axis
KC_E = (K_HALF * CP) // 128   # 10 contraction chunks for expand conv
KC_F = (2 * DIM) // 128       # 32 contraction chunks for fusion conv
BF16 = mybir.dt.bfloat16
F32 = mybir.dt.float32
E3M4 = mybir.dt.float8e3
# weights ship as fp8-e3m4 pre-scaled by a power of two (picked so the
# scaled values sit in e3m4's normal range); the inverse scale folds into
# the activation that drains each stage's psum.
WR_K = 16.0
WC_K = 64.0
WF_K = 64.0

_CACHE = {}


def _build():
    nc = bacc.Bacc("TRN2", target_bir_lowering=False, debug=False,
                   num_devices=N_CORES)
    x_d = nc.dram_tensor("x", [NT, 128, NSAMP * S], BF16, kind="ExternalInput")
    wr_d = nc.dram_tensor("wr", [2, 128, NT * CP], E3M4, kind="ExternalInput")
    # wc pre-paired on host: [b, pair, p, (mi kc m)] so each pair tile is one
    # fully-contiguous 327KB read
    wc_d = nc.dram_tensor("wc", [2, NT // 2, 128, 2 * KC_E * 128], E3M4,
                          kind="ExternalInput")
    wf_d = nc.dram_tensor("wf", [NT, 128, KC_F * 128], E3M4, kind="ExternalInput")
    out_d = nc.dram_tensor("out", [NT, 128, NSAMP * S], BF16, kind="ExternalOutput")

    AF = mybir.ActivationFunctionType
    OP = mybir.AluOpType

    with tile.TileContext(nc) as tc:
        with (
            tc.tile_pool(name="big", bufs=1) as big,
            tc.tile_pool(name="wcp", bufs=10) as wcp,
            tc.tile_pool(name="wfp", bufs=16) as wfp,
            tc.tile_pool(name="op", bufs=2) as op,
            tc.tile_pool(name="sgp", bufs=2) as sgp,
            tc.tile_pool(name="ps", bufs=8, space="PSUM") as ps,
        ):
            x_sb = big.tile([128, NT * NSAMP * S], BF16, name="x_sb", tag="x")
            wr_sb = [big.tile([128, NT * CP], E3M4, name=f"wr_sb{b}", tag=f"wr{b}")
                     for b in range(2)]
            xs_sb = [big.tile([128, 2 * K_HALF * S], BF16, name=f"xs_sb{b}", tag=f"xs{b}") for b in range(2)]
            s_sb = big.tile([128, KC_F * S], BF16, name="s_sb", tag="s")
            g_sb = big.tile([128, NT * S], BF16, name="g_sb", tag="g")

            # ---- input DMAs: one ring (sync), FIFO order == arrival ----
            # wr first (halved so the PE warm-up and R's first matmuls start
            # ~2us after the preamble), then the 16 x chunks (stage R streams
            # against them), then the wc pairs (pool slots 10..15 rotation-
            # block the sequencer briefly against stage E's frees — harmless,
            # only wf's triggers wait behind them and wf has ~40us of slack),
            # then wf.  Stores ride the OTHER HWDGE ring (scalar) so they
            # never queue behind loads in the engines' ring FIFOs.
            HR = NT * CP // 2
            nc.sync.dma_start(wr_sb[0][:, 0:HR], wr_d[0, :, 0:HR])
            nc.sync.dma_start(wr_sb[0][:, HR:], wr_d[0, :, HR:])
            nc.sync.dma_start(
                x_sb[:, 0:NSAMP * S], x_d[0, :, :])
            nc.sync.dma_start(wr_sb[1][:], wr_d[1, :, :])
            for kc in range(1, NT):
                nc.sync.dma_start(
                    x_sb[:, kc * NSAMP * S:(kc + 1) * NSAMP * S],
                    x_d[kc, :, :])
            wc_tiles = {}
            for b in range(2):
                for mc0 in range(0, NT, 2):
                    wct = wcp.tile([128, 2 * KC_E * 128], E3M4, name="wct", tag="wct")
                    nc.sync.dma_start(wct[:], wc_d[b, mc0 // 2, :, :])
                    wc_tiles[(b, mc0)] = wct
            wf_tiles = {}
            for k in range(NT):
                wft = wfp.tile([128, KC_F * 128], E3M4, name="wft", tag="wft")
                nc.sync.dma_start(wft[:], wf_d[k, :, :])
                wf_tiles[k] = wft

            # Prime the scalar engine's sigmoid activation table at the head
            # so stage F's first sigmoid doesn't stall on a table swap.
            sgd = big.tile([128, 1], BF16, name="sgd", tag="sgd")
            nc.vector.memset(sgd[:], 0.0)
            nc.scalar.activation(sgd[:], sgd[:], AF.Sigmoid)

            # PE warm-up on a memset tile — no DMA dependency, so it ramps
            # the PE clock during the preamble instead of waiting for wr.
            # Results are discarded.
            wmt = big.tile([128, 512], BF16, name="wmt", tag="wmt")
            nc.vector.memset(wmt[:], 0.0)
            wpt = ps.tile([128, 512], F32, name="wpt", tag="pt")
            for i in range(6):
                nc.tensor.matmul(wpt[:], wmt[:, 0:128], wmt[:],
                                 start=(i == 0), stop=(i == 5))

            # Stage R: reduce conv, xs = Wr @ x, straight off the bf16 x.
            # 8 psum groups of N=480 (2.5 samples each); kc-inner streams
            # against the x chunk DMAs via subtile deps.
            NH = K_HALF * S // 2          # 480 cols per psum group
            rpt = {}
            for b in range(2):
                for mc in range(2):
                    for h in range(2):
                        rpt[(b, mc, h)] = ps.tile([128, NH], F32, name="pt", tag="pt")
            for kc in range(NT):
                for b in range(2):
                    for mc in range(2):
                        for h in range(2):
                            col = kc * NSAMP * S + b * K_HALF * S + h * NH
                            nc.tensor.matmul(
                                rpt[(b, mc, h)][:],
                                wr_sb[b][:, kc * CP + mc * 128: kc * CP + (mc + 1) * 128],
                                x_sb[:, col: col + NH],
                                start=(kc == 0), stop=(kc == NT - 1))
            # scatter psum cols (2.5 samples per group) into xs channel-stack,
            # all on the scalar engine (Copy doesn't swap its Sigmoid table);
            # the vector engine stays free for stage E's relu drains.
            for b in range(2):
                for mc in range(2):
                    for h in range(2):
                        base_col = h * NH          # within branch b's 5 samples
                        off = 0
                        while off < NH:
                            j = (base_col + off) // S
                            joff = (base_col + off) % S
                            seg = min(S - joff, NH - off)
                            nc.scalar.activation(
                                xs_sb[b][:, (2 * j + mc) * S + joff:
                                       (2 * j + mc) * S + joff + seg],
                                rpt[(b, mc, h)][:, off:off + seg],
                                AF.Copy, scale=1.0 / WR_K)
                            off += seg

            # Stage E: expand conv, s = relu(Wc @ xs-stack); wc loaded in
            # paired slices for bigger DMAs
            for b in range(2):
                for mc0 in range(0, NT, 2):
                    wct = wc_tiles[(b, mc0)]
                    for mi in range(2):
                        mc = mc0 + mi
                        pt = ps.tile([128, S], F32, name="pt", tag="pt")
                        for kc in range(KC_E):
                            nc.tensor.matmul(
                                pt[:],
                                wct[:, (mi * KC_E + kc) * 128:(mi * KC_E + kc + 1) * 128],
                                xs_sb[b][:, kc * S:(kc + 1) * S],
                                start=(kc == 0), stop=(kc == KC_E - 1))
                        # scaled relu on the (otherwise idle) vector engine,
                        # keeping the scalar engine's activation table on
                        # Sigmoid for stage F
                        nc.vector.tensor_scalar(
                            s_sb[:, (b * NT + mc) * S:(b * NT + mc + 1) * S],
                            pt[:], 1.0 / WC_K, 0.0,
                            OP.mult, OP.max)

            # Stage F+O: fusion conv + sigmoid, then out = x * (1 + sig).
            # Stores are full chunks (3840B contiguous per partition — line
            # rate) on the scalar HWDGE ring, so they start the moment their
            # gate finishes regardless of load-queue state; the tail past the
            # last matmul is one chunk's sigmoid+gate+store (~3us).
            for mc in range(NT):
                wft = wf_tiles[mc]
                pt = ps.tile([128, S], F32, name="pt", tag="pt")
                for kc in range(KC_F):
                    nc.tensor.matmul(
                        pt[:],
                        wft[:, kc * 128:(kc + 1) * 128],
                        s_sb[:, kc * S:(kc + 1) * S],
                        start=(kc == 0), stop=(kc == KC_F - 1))
                sgt = sgp.tile([128, S], BF16, name="sgt", tag="sgt")
                nc.scalar.activation(sgt[:], pt[:], AF.Sigmoid, scale=1.0 / WF_K)
                # g = 1 + sig on the scalar engine (Copy keeps its table)
                nc.scalar.activation(g_sb[:, mc * S:(mc + 1) * S],
                                     sgt[:], AF.Copy, bias=1.0)
                # gate: out = x * g, g broadcast over the chunk's 10 samples
                ot = op.tile([128, NSAMP * S], BF16, name="ot", tag="ot")
                g_b, x_b = bass.broadcast_tensor_aps(
                    g_sb[:, mc * S:(mc + 1) * S].rearrange("p (j s) -> p j s", j=1),
                    x_sb[:, mc * NSAMP * S:(mc + 1) * NSAMP * S]
                        .rearrange("p (j s) -> p j s", j=NSAMP))
                nc.vector.tensor_tensor(
                    ot[:].rearrange("p (j s) -> p j s", j=NSAMP),
                    x_b, g_b, OP.mult)
                if mc == NT - 1:
                    # final chunk split across both HWDGE rings to burst out
                    HS = NSAMP * S // 2
                    nc.scalar.dma_start(out_d[mc, :, 0:HS], ot[:, 0:HS])
                    nc.sync.dma_start(out_d[mc, :, HS:], ot[:, HS:])
                else:
                    nc.scalar.dma_start(out_d[mc, :, :], ot[:])

    nc.compile()
    return nc


def _get_nc():
    if "nc" not in _CACHE:
        _CACHE["nc"] = _build()
    return _CACHE["nc"]


def _prep_weights(W_ri, W_rv, W_ci, W_cv, W_f):
    f8 = ml_dtypes.float8_e3m4
    # wr[b][p, kc*CP + m] = W_r[m, kc*128 + p], scaled into e3m4 range
    wr = np.stack([
        np.ascontiguousarray(
            W.T.reshape(NT, 128, CP).transpose(1, 0, 2).reshape(128, NT * CP))
        for W in (W_ri, W_rv)
    ])
    wr = (wr * WR_K).astype(f8)
    # wc[b][pair][p][mi*KC_E*128 + kc*128+m] = W_c[(2*pair+mi)*128+m, kc*128+p]
    # (pair-major so each [128, 2560] tile is one contiguous DMA)
    wc = np.stack([
        np.ascontiguousarray(
            W.reshape(NT // 2, 2, 128, KC_E, 128)      # pair mi m' kc p'
             .transpose(0, 4, 1, 3, 2)                 # pair p' mi kc m'
             .reshape(NT // 2, 128, 2 * KC_E * 128))
        for W in (W_ci, W_cv)
    ])
    wc = (wc * WC_K).astype(f8)
    # wf[mc][p][kc*128+m] = W_f[mc*128+m, kc*128+p]
    wf = np.ascontiguousarray(
        W_f.reshape(NT, 128, KC_F, 128).transpose(0, 3, 2, 1).reshape(NT, 128, KC_F * 128)
    )
    wf = (wf * WF_K).astype(f8)
    return wr, wc, wf


def kernel(inputs, labels, sub, W_ri, W_rv, W_ci, W_cv, W_f):
    inputs = np.asarray(inputs, dtype=np.float32)
    labels = np.asarray(labels)
    sub = np.asarray(sub)
    W_ri = np.asarray(W_ri, dtype=np.float32)
    W_rv = np.asarray(W_rv, dtype=np.float32)
    W_ci = np.asarray(W_ci, dtype=np.float32)
    W_cv = np.asarray(W_cv, dtype=np.float32)
    W_f = np.asarray(W_f, dtype=np.float32)

    n, c, h, w = inputs.shape
    assert (n, c, h * w) == (N_CORES * NSAMP, DIM, S)
    x = inputs.reshape(n, c, h * w)

    # identity groups: all samples of one label share the same gather set
    uniq = np.unique(labels)
    assert len(uniq) == N_CORES, f"expected {N_CORES} identities, got {len(uniq)}"
    order = []
    for g in uniq:
        idx = np.nonzero(labels == g)[0]
        i_s = [int(i) for i in idx if sub[i] == 1]
        v_s = [int(i) for i in idx if sub[i] == 0]
        assert len(i_s) == K_HALF and len(v_s) == K_HALF, \
            f"identity {g}: {len(i_s)}/{len(v_s)} split not {K_HALF}/{K_HALF}"
        order.append(i_s + v_s)

    wr, wc, wf = _prep_weights(W_ri, W_rv, W_ci, W_cv, W_f)

    bf = ml_dtypes.bfloat16
    in_maps = []
    for g in range(N_CORES):
        xg = x[order[g]]                                      # [10, 2048, 192]
        xt = np.ascontiguousarray(
            xg.reshape(NSAMP, NT, 128, S).transpose(1, 2, 0, 3)
              .reshape(NT, 128, NSAMP * S)).astype(bf)
        in_maps.append({"x": xt, "wr": wr, "wc": wc, "wf": wf})

    nc = _get_nc()
    res = run_bass_kernel_spmd(nc, in_maps, core_ids=list(range(N_CORES)))

    out = np.empty_like(inputs)
    for g in range(N_CORES):
        og = res.results[g]["out"]                            # [16, 128, 1920] bf16
        og = og.astype(np.float32)
        og = og.reshape(NT, 128, NSAMP, S).transpose(2, 0, 1, 3).reshape(NSAMP, c, h, w)
        out[order[g]] = og
    return out


# revision 14
# speedup vs baseline: 1.0063x; 1.0063x over previous
"""Trainium2 Bass kernel for the PK-batch message-passing gating module.

Reference computation (per full batch of N=80 samples, 8 identities x
(5 sub=1 + 5 sub=0) samples):
  for each branch b in {sub==1, sub==0}:
    xs   = Wr_b @ x[subgroup_b]                (1x1 conv 2048 -> 256)
    aff  = per-sample gather of the 5 same-label subgroup samples,
           channel-stacked -> 1280 channels
    s_b  = relu(Wc_b @ aff)                    (1x1 conv 1280 -> 2048)
  x_fuse = sigmoid(W_f @ concat(s_i, s_v))     (1x1 conv 4096 -> 2048)
  out    = inputs * (1 + x_fuse)

All samples of one identity share the same gather, hence the same
x_fuse — so the message passing is computed once per identity and the
sigmoid gate broadcast over that identity's 10 samples.  Sharding: one
identity per NeuronCore (8 identities / 8 cores, data parallel);
weights replicated, pre-transposed/tiled on host, fp8 e3m4.

x ships ONCE as bf16 and feeds both the stage-R matmuls (bf16 moving
data runs at the same 1 col/cycle as fp8) and the output gate — no
separate fp8 copy, which both cuts HBM traffic ~11% and removes the
x-quantization error.  The single-ring FIFO DMA order is the schedule:
wr + x chunks first (stage R streams against them), the first two
expand-conv pairs slipped in before the last x chunks so stage E is
never starved, then the rest of wc and all of wf — everything on-chip
before its stage begins, so the F window carries only output stores
and the store tail after the last matmul is just the final chunk.
"""

import numpy as np
import ml_dtypes

import concourse.bass as bass
import concourse.tile as tile
from concourse import bacc, mybir
from concourse.bass_utils import run_bass_kernel_spmd

N_CORES = 8
K_HALF = 5
NSAMP = 2 * K_HALF        # samples per identity
DIM = 2048
CP = 256                  # reduced channels per branch
S = 24 * 8                # spatial positions per sample
NT = DIM // 128           # 16 channel chunks of the 2048-dim axis
KC_E = (K_HALF * CP) // 128   # 10 contraction chunks for expand conv
KC_F = (2 * DIM) // 128       # 32 contraction chunks for fusion conv
BF16 = mybir.dt.bfloat16
F32 = mybir.dt.float32
E3M4 = mybir.dt.float8e3
E4M3 = mybir.dt.float8e4
# wc ships fp8-e3m4, wf fp8-e4m3 (DoubleRow needs e4/e5), wr bf16 —
# each pre-scaled by a power of two into the fp8 normal range; the
# inverse scales fold into the activations that drain each stage's psum.
# Stage F runs perf_mode=DoubleRow (2 fp8 mults/cell/cycle), which needs
# BOTH operands in e4m3: s is stored e4m3 scaled by S_K.
WC_K = 64.0
WF_K = 512.0
S_K = 4.0

_CACHE = {}


def _build():
    nc = bacc.Bacc("TRN2", target_bir_lowering=False, debug=False,
                   num_devices=N_CORES)
    x_d = nc.dram_tensor("x", [NT, 128, NSAMP * S], BF16, kind="ExternalInput")
    wr_d = nc.dram_tensor("wr", [2, 128, NT * CP], BF16, kind="ExternalInput")
    # wc pre-paired on host: [b, pair, p, (mi kc m)] so each pair tile is one
    # fully-contiguous 327KB read
    wc_d = nc.dram_tensor("wc", [2, NT // 2, 128, 2 * KC_E * 128], E3M4,
                          kind="ExternalInput")
    wf_d = nc.dram_tensor("wf", [NT, 128, KC_F * 128], E4M3, kind="ExternalInput")
    out_d = nc.dram_tensor("out", [NT, 128, NSAMP * S], BF16, kind="ExternalOutput")

    AF = mybir.ActivationFunctionType
    OP = mybir.AluOpType

    with tile.TileContext(nc) as tc:
        with (
            tc.tile_pool(name="big", bufs=1) as big,
            tc.tile_pool(name="wcp", bufs=10) as wcp,
            tc.tile_pool(name="wfp", bufs=16) as wfp,
            tc.tile_pool(name="op", bufs=2) as op,
            tc.tile_pool(name="sgp", bufs=2) as sgp,
            tc.tile_pool(name="ps", bufs=8, space="PSUM") as ps,
        ):
            x_sb = big.tile([128, NT * NSAMP * S], BF16, name="x_sb", tag="x")
            wr_sb = [big.tile([128, NT * CP], BF16, name=f"wr_sb{b}", tag=f"wr{b}")
                     for b in range(2)]
            xs_sb = [big.tile([128, 2 * K_HALF * S], BF16, name=f"xs_sb{b}", tag=f"xs{b}") for b in range(2)]
            s_sb = big.tile([128, KC_F * S], E4M3, name="s_sb", tag="s")
            g_sb = big.tile([128, NT * S], BF16, name="g_sb", tag="g")

            # ---- input DMAs: one ring (sync), FIFO order == arrival ----
            # wr first (halved so the PE warm-up and R's first matmuls start
            # ~2us after the preamble), then the 16 x chunks (stage R streams
            # against them), then the wc pairs (pool slots 10..15 rotation-
            # block the sequencer briefly against stage E's frees — harmless,
            # only wf's triggers wait behind them and wf has ~40us of slack),
            # then wf.  Stores ride the OTHER HWDGE ring (scalar) so they
            # never queue behind loads in the engines' ring FIFOs.
            HR = NT * CP // 2
            nc.sync.dma_start(wr_sb[0][:, 0:HR], wr_d[0, :, 0:HR])
            nc.sync.dma_start(wr_sb[0][:, HR:], wr_d[0, :, HR:])
            nc.sync.dma_start(
                x_sb[:, 0:NSAMP * S], x_d[0, :, :])
            nc.sync.dma_start(wr_sb[1][:], wr_d[1, :, :])
            for kc in range(1, NT):
                nc.sync.dma_start(
                    x_sb[:, kc * NSAMP * S:(kc + 1) * NSAMP * S],
                    x_d[kc, :, :])
            wc_tiles = {}
            for b in range(2):
                for mc0 in range(0, NT, 2):
                    wct = wcp.tile([128, 2 * KC_E * 128], E3M4, name="wct", tag="wct")
                    nc.sync.dma_start(wct[:], wc_d[b, mc0 // 2, :, :])
                    wc_tiles[(b, mc0)] = wct
            wf_tiles = {}
            for k in range(NT):
                wft = wfp.tile([128, KC_F * 128], E4M3, name="wft", tag="wft")
                nc.sync.dma_start(wft[:], wf_d[k, :, :])
                wf_tiles[k] = wft

            # Prime the scalar engine's sigmoid activation table at the head
            # so stage F's first sigmoid doesn't stall on a table swap.
            sgd = big.tile([128, 1], BF16, name="sgd", tag="sgd")
            nc.vector.memset(sgd[:], 0.0)
            nc.scalar.activation(sgd[:], sgd[:], AF.Sigmoid)

            # PE warm-up on a memset tile — no DMA dependency, so it ramps
            # the PE clock during the preamble instead of waiting for wr.
            # Results are discarded.
            wmt = big.tile([128, 512], BF16, name="wmt", tag="wmt")
            nc.vector.memset(wmt[:], 0.0)
            wpt = ps.tile([128, 512], F32, name="wpt", tag="pt")
            for i in range(6):
                nc.tensor.matmul(wpt[:], wmt[:, 0:128], wmt[:],
                                 start=(i == 0), stop=(i == 5))

            # Stage R: reduce conv, xs = Wr @ x, straight off the bf16 x.
            # 8 psum groups of N=480 (2.5 samples each); kc-inner streams
            # against the x chunk DMAs via subtile deps.
            NH = K_HALF * S // 2          # 480 cols per psum group
            rpt = {}
            for b in range(2):
                for mc in range(2):
                    for h in range(2):
                        rpt[(b, mc, h)] = ps.tile([128, NH], F32, name="pt", tag="pt")
            for kc in range(NT):
                for b in range(2):
                    for mc in range(2):
                        for h in range(2):
                            col = kc * NSAMP * S + b * K_HALF * S + h * NH
                            nc.tensor.matmul(
                                rpt[(b, mc, h)][:],
                                wr_sb[b][:, kc * CP + mc * 128: kc * CP + (mc + 1) * 128],
                                x_sb[:, col: col + NH],
                                start=(kc == 0), stop=(kc == NT - 1))
            # scatter psum cols (2.5 samples per group) into xs channel-stack,
            # all on the scalar engine (Copy doesn't swap its Sigmoid table);
            # the vector engine stays free for stage E's relu drains.
            for b in range(2):
                for mc in range(2):
                    for h in range(2):
                        base_col = h * NH          # within branch b's 5 samples
                        off = 0
                        while off < NH:
                            j = (base_col + off) // S
                            joff = (base_col + off) % S
                            seg = min(S - joff, NH - off)
                            nc.scalar.activation(
                                xs_sb[b][:, (2 * j + mc) * S + joff:
                                       (2 * j + mc) * S + joff + seg],
                                rpt[(b, mc, h)][:, off:off + seg],
                                AF.Copy)
                            off += seg

            # Stage E: expand conv, s = relu(Wc @ xs-stack); wc loaded in
            # paired slices for bigger DMAs
            for b in range(2):
                for mc0 in range(0, NT, 2):
                    wct = wc_tiles[(b, mc0)]
                    for mi in range(2):
                        mc = mc0 + mi
                        pt = ps.tile([128, S], F32, name="pt", tag="pt")
                        for kc in range(KC_E):
                            nc.tensor.matmul(
                                pt[:],
                                wct[:, (mi * KC_E + kc) * 128:(mi * KC_E + kc + 1) * 128],
                                xs_sb[b][:, kc * S:(kc + 1) * S],
                                start=(kc == 0), stop=(kc == KC_E - 1))
                        # scaled relu on the (otherwise idle) vector engine,
                        # keeping the scalar engine's activation table on
                        # Sigmoid for stage F
                        nc.vector.tensor_scalar(
                            s_sb[:, (b * NT + mc) * S:(b * NT + mc + 1) * S],
                            pt[:], S_K / WC_K, 0.0,
                            OP.mult, OP.max)

            # Stage F+O: fusion conv + sigmoid, then out = x * (1 + sig).
            # Stores are full chunks (3840B contiguous per partition — line
            # rate) on the scalar HWDGE ring, so they start the moment their
            # gate finishes regardless of load-queue state; the tail past the
            # last matmul is one chunk's sigmoid+gate+store (~3us).
            for mc in range(NT):
                wft = wf_tiles[mc]
                pt = ps.tile([128, S], F32, name="pt", tag="pt")
                for t in range(KC_F // 2):
                    nc.tensor.matmul(
                        pt[:],
                        wft[:, 2 * t * 128:(2 * t + 2) * 128]
                            .rearrange("p (two m) -> p two m", two=2),
                        s_sb[:, 2 * t * S:(2 * t + 2) * S]
                            .rearrange("p (two n) -> p two n", two=2),
                        start=(t == 0), stop=(t == KC_F // 2 - 1),
                        perf_mode=mybir.MatmulPerfMode.DoubleRow)
                sgt = sgp.tile([128, S], BF16, name="sgt", tag="sgt")
                nc.scalar.activation(sgt[:], pt[:], AF.Sigmoid,
                                     scale=1.0 / (WF_K * S_K))
                # g = 1 + sig on the scalar engine (Copy keeps its table)
                nc.scalar.activation(g_sb[:, mc * S:(mc + 1) * S],
                                     sgt[:], AF.Copy, bias=1.0)
                # gate: out = x * g, g broadcast over the chunk's 10 samples
                ot = op.tile([128, NSAMP * S], BF16, name="ot", tag="ot")
                g_b, x_b = bass.broadcast_tensor_aps(
                    g_sb[:, mc * S:(mc + 1) * S].rearrange("p (j s) -> p j s", j=1),
                    x_sb[:, mc * NSAMP * S:(mc + 1) * NSAMP * S]
                        .rearrange("p (j s) -> p j s", j=NSAMP))
                nc.vector.tensor_tensor(
                    ot[:].rearrange("p (j s) -> p j s", j=NSAMP),
                    x_b, g_b, OP.mult)
                if mc == NT - 1:
                    # final chunk split across both HWDGE rings to burst out
                    HS = NSAMP * S // 2
                    nc.scalar.dma_start(out_d[mc, :, 0:HS], ot[:, 0:HS])
                    nc.sync.dma_start(out_d[mc, :, HS:], ot[:, HS:])
                else:
                    nc.scalar.dma_start(out_d[mc, :, :], ot[:])

    nc.compile()
    return nc


def _get_nc():
    if "nc" not in _CACHE:
        _CACHE["nc"] = _build()
    return _CACHE["nc"]


def _prep_weights(W_ri, W_rv, W_ci, W_cv, W_f):
    f8 = ml_dtypes.float8_e3m4
    # wr[b][p, kc*CP + m] = W_r[m, kc*128 + p], bf16 unscaled
    wr = np.stack([
        np.ascontiguousarray(
            W.T.reshape(NT, 128, CP).transpose(1, 0, 2).reshape(128, NT * CP))
        for W in (W_ri, W_rv)
    ])
    wr = wr.astype(ml_dtypes.bfloat16)
    # wc[b][pair][p][mi*KC_E*128 + kc*128+m] = W_c[(2*pair+mi)*128+m, kc*128+p]
    # (pair-major so each [128, 2560] tile is one contiguous DMA)
    wc = np.stack([
        np.ascontiguousarray(
            W.reshape(NT // 2, 2, 128, KC_E, 128)      # pair mi m' kc p'
             .transpose(0, 4, 1, 3, 2)                 # pair p' mi kc m'
             .reshape(NT // 2, 128, 2 * KC_E * 128))
        for W in (W_ci, W_cv)
    ])
    wc = (wc * WC_K).astype(f8)
    # wf[mc][p][kc*128+m] = W_f[mc*128+m, kc*128+p]
    wf = np.ascontiguousarray(
        W_f.reshape(NT, 128, KC_F, 128).transpose(0, 3, 2, 1).reshape(NT, 128, KC_F * 128)
    )
    wf = (wf * WF_K).astype(ml_dtypes.float8_e4m3)
    return wr, wc, wf


def kernel(inputs, labels, sub, W_ri, W_rv, W_ci, W_cv, W_f):
    inputs = np.asarray(inputs, dtype=np.float32)
    labels = np.asarray(labels)
    sub = np.asarray(sub)
    W_ri = np.asarray(W_ri, dtype=np.float32)
    W_rv = np.asarray(W_rv, dtype=np.float32)
    W_ci = np.asarray(W_ci, dtype=np.float32)
    W_cv = np.asarray(W_cv, dtype=np.float32)
    W_f = np.asarray(W_f, dtype=np.float32)

    n, c, h, w = inputs.shape
    assert (n, c, h * w) == (N_CORES * NSAMP, DIM, S)
    x = inputs.reshape(n, c, h * w)

    # identity groups: all samples of one label share the same gather set
    uniq = np.unique(labels)
    assert len(uniq) == N_CORES, f"expected {N_CORES} identities, got {len(uniq)}"
    order = []
    for g in uniq:
        idx = np.nonzero(labels == g)[0]
        i_s = [int(i) for i in idx if sub[i] == 1]
        v_s = [int(i) for i in idx if sub[i] == 0]
        assert len(i_s) == K_HALF and len(v_s) == K_HALF, \
            f"identity {g}: {len(i_s)}/{len(v_s)} split not {K_HALF}/{K_HALF}"
        order.append(i_s + v_s)

    wr, wc, wf = _prep_weights(W_ri, W_rv, W_ci, W_cv, W_f)

    bf = ml_dtypes.bfloat16
    in_maps = []
    for g in range(N_CORES):
        xg = x[order[g]]                                      # [10, 2048, 192]
        xt = np.ascontiguousarray(
            xg.reshape(NSAMP, NT, 128, S).transpose(1, 2, 0, 3)
              .reshape(NT, 128, NSAMP * S)).astype(bf)
        in_maps.append({"x": xt, "wr": wr, "wc": wc, "wf": wf})

    nc = _get_nc()
    res = run_bass_kernel_spmd(nc, in_maps, core_ids=list(range(N_CORES)))

    out = np.empty_like(inputs)
    for g in range(N_CORES):
        og = res.results[g]["out"]                            # [16, 128, 1920] bf16
        og = og.astype(np.float32)
        og = og.reshape(NT, 128, NSAMP, S).transpose(2, 0, 1, 3).reshape(NSAMP, c, h, w)
        out[order[g]] = og
    return out


# revision 16
# speedup vs baseline: 1.0726x; 1.0659x over previous
"""Trainium2 Bass kernel for the PK-batch message-passing gating module.

Reference computation (per full batch of N=80 samples, 8 identities x
(5 sub=1 + 5 sub=0) samples):
  for each branch b in {sub==1, sub==0}:
    xs   = Wr_b @ x[subgroup_b]                (1x1 conv 2048 -> 256)
    aff  = per-sample gather of the 5 same-label subgroup samples,
           channel-stacked -> 1280 channels
    s_b  = relu(Wc_b @ aff)                    (1x1 conv 1280 -> 2048)
  x_fuse = sigmoid(W_f @ concat(s_i, s_v))     (1x1 conv 4096 -> 2048)
  out    = inputs * (1 + x_fuse)

All samples of one identity share the same gather, hence the same
x_fuse — so the message passing is computed once per identity and the
sigmoid gate broadcast over that identity's 10 samples.  Sharding: one
identity per NeuronCore (8 identities / 8 cores, data parallel);
weights replicated, pre-transposed/tiled on host, fp8 e3m4.

x ships ONCE as bf16 and feeds both the stage-R matmuls (bf16 moving
data runs at the same 1 col/cycle as fp8) and the output gate — no
separate fp8 copy, which both cuts HBM traffic ~11% and removes the
x-quantization error.  The single-ring FIFO DMA order is the schedule:
wr + x chunks first (stage R streams against them), the first two
expand-conv pairs slipped in before the last x chunks so stage E is
never starved, then the rest of wc and all of wf — everything on-chip
before its stage begins, so the F window carries only output stores
and the store tail after the last matmul is just the final chunk.
"""

import numpy as np
import ml_dtypes

import concourse.bass as bass
import concourse.tile as tile
from concourse import bacc, mybir
from concourse.bass_utils import run_bass_kernel_spmd

N_CORES = 8
K_HALF = 5
NSAMP = 2 * K_HALF        # samples per identity
DIM = 2048
CP = 256                  # reduced channels per branch
S = 24 * 8                # spatial positions per sample
NT = DIM // 128           # 16 channel chunks of the 2048-dim axis
KC_E = (K_HALF * CP) // 128   # 10 contraction chunks for expand conv
KC_F = (2 * DIM) // 128       # 32 contraction chunks for fusion conv
BF16 = mybir.dt.bfloat16
F32 = mybir.dt.float32
E3M4 = mybir.dt.float8e3
E4M3 = mybir.dt.float8e4
# wc ships fp8-e3m4, wf fp8-e4m3 (DoubleRow needs e4/e5), wr bf16 —
# each pre-scaled by a power of two into the fp8 normal range; the
# inverse scales fold into the activations that drain each stage's psum.
# Stage F runs perf_mode=DoubleRow (2 fp8 mults/cell/cycle), which needs
# BOTH operands in e4m3: s is stored e4m3 scaled by S_K.
WC_K = 64.0
WF_K = 512.0
S_K = 4.0

_CACHE = {}


def _build():
    nc = bacc.Bacc("TRN2", target_bir_lowering=False, debug=False,
                   num_devices=N_CORES)
    x_d = nc.dram_tensor("x", [NT, 128, NSAMP * S], BF16, kind="ExternalInput")
    wr_d = nc.dram_tensor("wr", [2, 128, NT * CP], BF16, kind="ExternalInput")
    # wc pre-paired on host: [b, pair, p, (mi kc m)] so each pair tile is one
    # fully-contiguous 327KB read
    wc_d = nc.dram_tensor("wc", [2, NT // 2, 128, 2 * KC_E * 128], E3M4,
                          kind="ExternalInput")
    wf_d = nc.dram_tensor("wf", [NT, 128, KC_F * 128], E4M3, kind="ExternalInput")
    out_d = nc.dram_tensor("out", [NT, 128, NSAMP * S], BF16, kind="ExternalOutput")

    AF = mybir.ActivationFunctionType
    OP = mybir.AluOpType

    with tile.TileContext(nc) as tc:
        with (
            tc.tile_pool(name="big", bufs=1) as big,
            tc.tile_pool(name="wcp", bufs=13) as wcp,
            tc.tile_pool(name="wfp", bufs=16) as wfp,
            tc.tile_pool(name="op", bufs=2) as op,
            tc.tile_pool(name="sgp", bufs=2) as sgp,
            tc.tile_pool(name="ps", bufs=8, space="PSUM") as ps,
        ):
            x_sb = big.tile([128, NT * NSAMP * S], BF16, name="x_sb", tag="x")
            wr_sb = [big.tile([128, NT * CP], BF16, name=f"wr_sb{b}", tag=f"wr{b}")
                     for b in range(2)]
            xs_sb = [big.tile([128, 2 * K_HALF * S], BF16, name=f"xs_sb{b}", tag=f"xs{b}") for b in range(2)]
            s_sb = big.tile([128, KC_F * S], E4M3, name="s_sb", tag="s")
            g_sb = big.tile([128, NT * S], BF16, name="g_sb", tag="g")

            # ---- input DMAs: one ring (sync), FIFO order == arrival ----
            # wr first (halved so the PE warm-up and R's first matmuls start
            # ~2us after the preamble), then the 16 x chunks (stage R streams
            # against them), then the wc pairs (pool slots 10..15 rotation-
            # block the sequencer briefly against stage E's frees — harmless,
            # only wf's triggers wait behind them and wf has ~40us of slack),
            # then wf.  Stores ride the OTHER HWDGE ring (scalar) so they
            # never queue behind loads in the engines' ring FIFOs.
            # wr quartered and interleaved with the first x chunks: R's kc=0
            # matmuls need only wr's first quarter (cols 0:1024 cover kc<4),
            # so R starts ~4us earlier than with whole-wr-first.
            QR = NT * CP // 4
            nc.sync.dma_start(wr_sb[0][:, 0:QR], wr_d[0, :, 0:QR])
            nc.sync.dma_start(
                x_sb[:, 0:NSAMP * S], x_d[0, :, :])
            nc.sync.dma_start(wr_sb[1][:, 0:QR], wr_d[1, :, 0:QR])
            nc.sync.dma_start(
                x_sb[:, NSAMP * S:2 * NSAMP * S], x_d[1, :, :])
            nc.sync.dma_start(wr_sb[0][:, QR:2 * QR], wr_d[0, :, QR:2 * QR])
            nc.sync.dma_start(wr_sb[1][:, QR:2 * QR], wr_d[1, :, QR:2 * QR])
            nc.sync.dma_start(
                x_sb[:, 2 * NSAMP * S:3 * NSAMP * S], x_d[2, :, :])
            nc.sync.dma_start(wr_sb[0][:, 2 * QR:], wr_d[0, :, 2 * QR:])
            nc.sync.dma_start(wr_sb[1][:, 2 * QR:], wr_d[1, :, 2 * QR:])
            for kc in range(3, NT):
                nc.sync.dma_start(
                    x_sb[:, kc * NSAMP * S:(kc + 1) * NSAMP * S],
                    x_d[kc, :, :])
            wc_tiles = {}
            for b in range(2):
                for mc0 in range(0, NT, 2):
                    wct = wcp.tile([128, 2 * KC_E * 128], E3M4, name="wct", tag="wct")
                    nc.sync.dma_start(wct[:], wc_d[b, mc0 // 2, :, :])
                    wc_tiles[(b, mc0)] = wct
            wf_tiles = {}
            for k in range(NT):
                wft = wfp.tile([128, KC_F * 128], E4M3, name="wft", tag="wft")
                nc.sync.dma_start(wft[:], wf_d[k, :, :])
                wf_tiles[k] = wft

            # Prime the scalar engine's sigmoid activation table at the head
            # so stage F's first sigmoid doesn't stall on a table swap.
            sgd = big.tile([128, 1], BF16, name="sgd", tag="sgd")
            nc.vector.memset(sgd[:], 0.0)
            nc.scalar.activation(sgd[:], sgd[:], AF.Sigmoid)

            # PE warm-up on a memset tile — no DMA dependency, so it ramps
            # the PE clock during the preamble instead of waiting for wr.
            # Results are discarded.
            wmt = big.tile([128, 512], BF16, name="wmt", tag="wmt")
            nc.vector.memset(wmt[:], 0.0)
            wpt = ps.tile([128, 512], F32, name="wpt", tag="pt")
            for i in range(6):
                nc.tensor.matmul(wpt[:], wmt[:, 0:128], wmt[:],
                                 start=(i == 0), stop=(i == 5))

            # Stage R: reduce conv, xs = Wr @ x, straight off the bf16 x.
            # 8 psum groups of N=480 (2.5 samples each); kc-inner streams
            # against the x chunk DMAs via subtile deps.
            NH = K_HALF * S // 2          # 480 cols per psum group
            rpt = {}
            for b in range(2):
                for mc in range(2):
                    for h in range(2):
                        rpt[(b, mc, h)] = ps.tile([128, NH], F32, name="pt", tag="pt")
            for kc in range(NT):
                for b in range(2):
                    for mc in range(2):
                        for h in range(2):
                            col = kc * NSAMP * S + b * K_HALF * S + h * NH
                            nc.tensor.matmul(
                                rpt[(b, mc, h)][:],
                                wr_sb[b][:, kc * CP + mc * 128: kc * CP + (mc + 1) * 128],
                                x_sb[:, col: col + NH],
                                start=(kc == 0), stop=(kc == NT - 1))
            # scatter psum cols (2.5 samples per group) into xs channel-stack,
            # all on the scalar engine (Copy doesn't swap its Sigmoid table);
            # the vector engine stays free for stage E's relu drains.
            for b in range(2):
                for mc in range(2):
                    for h in range(2):
                        base_col = h * NH          # within branch b's 5 samples
                        off = 0
                        while off < NH:
                            j = (base_col + off) // S
                            joff = (base_col + off) % S
                            seg = min(S - joff, NH - off)
                            nc.scalar.activation(
                                xs_sb[b][:, (2 * j + mc) * S + joff:
                                       (2 * j + mc) * S + joff + seg],
                                rpt[(b, mc, h)][:, off:off + seg],
                                AF.Copy)
                            off += seg

            # Stage E: expand conv, s = relu(Wc @ xs-stack); wc loaded in
            # paired slices for bigger DMAs
            for b in range(2):
                for mc0 in range(0, NT, 2):
                    wct = wc_tiles[(b, mc0)]
                    for mi in range(2):
                        mc = mc0 + mi
                        pt = ps.tile([128, S], F32, name="pt", tag="pt")
                        for kc in range(KC_E):
                            nc.tensor.matmul(
                                pt[:],
                                wct[:, (mi * KC_E + kc) * 128:(mi * KC_E + kc + 1) * 128],
                                xs_sb[b][:, kc * S:(kc + 1) * S],
                                start=(kc == 0), stop=(kc == KC_E - 1))
                        # scaled relu on the (otherwise idle) vector engine,
                        # keeping the scalar engine's activation table on
                        # Sigmoid for stage F
                        nc.vector.tensor_scalar(
                            s_sb[:, (b * NT + mc) * S:(b * NT + mc + 1) * S],
                            pt[:], S_K / WC_K, 0.0,
                            OP.mult, OP.max)

            # Stage F+O: fusion conv + sigmoid, then out = x * (1 + sig).
            # Stores are full chunks (3840B contiguous per partition — line
            # rate) on the scalar HWDGE ring, so they start the moment their
            # gate finishes regardless of load-queue state; the tail past the
            # last matmul is one chunk's sigmoid+gate+store (~3us).
            for mc in range(NT):
                wft = wf_tiles[mc]
                pt = ps.tile([128, S], F32, name="pt", tag="pt")
                for t in range(KC_F // 2):
                    nc.tensor.matmul(
                        pt[:],
                        wft[:, 2 * t * 128:(2 * t + 2) * 128]
                            .rearrange("p (two m) -> p two m", two=2),
                        s_sb[:, 2 * t * S:(2 * t + 2) * S]
                            .rearrange("p (two n) -> p two n", two=2),
                        start=(t == 0), stop=(t == KC_F // 2 - 1),
                        perf_mode=mybir.MatmulPerfMode.DoubleRow)
                sgt = sgp.tile([128, S], BF16, name="sgt", tag="sgt")
                nc.scalar.activation(sgt[:], pt[:], AF.Sigmoid,
                                     scale=1.0 / (WF_K * S_K))
                # g = 1 + sig on the scalar engine (Copy keeps its table)
                nc.scalar.activation(g_sb[:, mc * S:(mc + 1) * S],
                                     sgt[:], AF.Copy, bias=1.0)
                # gate: out = x * g, g broadcast over the chunk's 10 samples
                ot = op.tile([128, NSAMP * S], BF16, name="ot", tag="ot")
                g_b, x_b = bass.broadcast_tensor_aps(
                    g_sb[:, mc * S:(mc + 1) * S].rearrange("p (j s) -> p j s", j=1),
                    x_sb[:, mc * NSAMP * S:(mc + 1) * NSAMP * S]
                        .rearrange("p (j s) -> p j s", j=NSAMP))
                nc.vector.tensor_tensor(
                    ot[:].rearrange("p (j s) -> p j s", j=NSAMP),
                    x_b, g_b, OP.mult)
                if mc == NT - 1:
                    # final chunk split across both HWDGE rings to burst out
                    HS = NSAMP * S // 2
                    nc.scalar.dma_start(out_d[mc, :, 0:HS], ot[:, 0:HS])
                    nc.sync.dma_start(out_d[mc, :, HS:], ot[:, HS:])
                else:
                    nc.scalar.dma_start(out_d[mc, :, :], ot[:])

    nc.compile()
    return nc


def _get_nc():
    if "nc" not in _CACHE:
        _CACHE["nc"] = _build()
    return _CACHE["nc"]


def _prep_weights(W_ri, W_rv, W_ci, W_cv, W_f):
    f8 = ml_dtypes.float8_e3m4
    # wr[b][p, kc*CP + m] = W_r[m, kc*128 + p], bf16 unscaled
    wr = np.stack([
        np.ascontiguousarray(
            W.T.reshape(NT, 128, CP).transpose(1, 0, 2).reshape(128, NT * CP))
        for W in (W_ri, W_rv)
    ])
    wr = wr.astype(ml_dtypes.bfloat16)
    # wc[b][pair][p][mi*KC_E*128 + kc*128+m] = W_c[(2*pair+mi)*128+m, kc*128+p]
    # (pair-major so each [128, 2560] tile is one contiguous DMA)
    wc = np.stack([
        np.ascontiguousarray(
            W.reshape(NT // 2, 2, 128, KC_E, 128)      # pair mi m' kc p'
             .transpose(0, 4, 1, 3, 2)                 # pair p' mi kc m'
             .reshape(NT // 2, 128, 2 * KC_E * 128))
        for W in (W_ci, W_cv)
    ])
    wc = (wc * WC_K).astype(f8)
    # wf[mc][p][kc*128+m] = W_f[mc*128+m, kc*128+p]
    wf = np.ascontiguousarray(
        W_f.reshape(NT, 128, KC_F, 128).transpose(0, 3, 2, 1).reshape(NT, 128, KC_F * 128)
    )
    wf = (wf * WF_K).astype(ml_dtypes.float8_e4m3)
    return wr, wc, wf


def kernel(inputs, labels, sub, W_ri, W_rv, W_ci, W_cv, W_f):
    inputs = np.asarray(inputs, dtype=np.float32)
    labels = np.asarray(labels)
    sub = np.asarray(sub)
    W_ri = np.asarray(W_ri, dtype=np.float32)
    W_rv = np.asarray(W_rv, dtype=np.float32)
    W_ci = np.asarray(W_ci, dtype=np.float32)
    W_cv = np.asarray(W_cv, dtype=np.float32)
    W_f = np.asarray(W_f, dtype=np.float32)

    n, c, h, w = inputs.shape
    assert (n, c, h * w) == (N_CORES * NSAMP, DIM, S)
    x = inputs.reshape(n, c, h * w)

    # identity groups: all samples of one label share the same gather set
    uniq = np.unique(labels)
    assert len(uniq) == N_CORES, f"expected {N_CORES} identities, got {len(uniq)}"
    order = []
    for g in uniq:
        idx = np.nonzero(labels == g)[0]
        i_s = [int(i) for i in idx if sub[i] == 1]
        v_s = [int(i) for i in idx if sub[i] == 0]
        assert len(i_s) == K_HALF and len(v_s) == K_HALF, \
            f"identity {g}: {len(i_s)}/{len(v_s)} split not {K_HALF}/{K_HALF}"
        order.append(i_s + v_s)

    wr, wc, wf = _prep_weights(W_ri, W_rv, W_ci, W_cv, W_f)

    bf = ml_dtypes.bfloat16
    in_maps = []
    for g in range(N_CORES):
        xg = x[order[g]]                                      # [10, 2048, 192]
        xt = np.ascontiguousarray(
            xg.reshape(NSAMP, NT, 128, S).transpose(1, 2, 0, 3)
              .reshape(NT, 128, NSAMP * S)).astype(bf)
        in_maps.append({"x": xt, "wr": wr, "wc": wc, "wf": wf})

    nc = _get_nc()
    res = run_bass_kernel_spmd(nc, in_maps, core_ids=list(range(N_CORES)))

    out = np.empty_like(inputs)
    for g in range(N_CORES):
        og = res.results[g]["out"]                            # [16, 128, 1920] bf16
        og = og.astype(np.float32)
        og = og.reshape(NT, 128, NSAMP, S).transpose(2, 0, 1, 3).reshape(NSAMP, c, h, w)
        out[order[g]] = og
    return out
